# revision 1
# baseline (speedup 1.0000x reference)
"""Bass/Tile TRN2 kernel for nn_BilateralCostVolume.

For each of 81 displacements d=(du,dv) and batch b:
    out[b,r,h,w] = <bilinear(f2n, p + (BM+d)), bilinear(f1n, p - (BM+d))> * mask
where f1n/f2n are channel-l2-normalized features, sampling matches
F.grid_sample(align_corners=False, border padding), and the zeros-padding
validity mask is binarized at 0.999.

Sharding: 162 (b, r) planes over 8 cores.  Slot positions have a STATIC
batch: slots 0..10 hold b=0 planes, slots 11..21 hold b=1 planes (padded with
duplicates), so each slot's gathers read a compile-time table tensor.

Per core:
  1. normalize both features for both batches, build 4 "quad tables" in DRAM:
     row (y,x) = [F[y,x], F[y,x+1], F[y+1,x], F[y+1,x+1]] (edge-clamped),
     192 f32 = 768 B per row;
  2. per plane: compute sample coords / bilinear weights / masks as
     [w=128, h=80] fields on DVE/ACT; build the wrapped int16 index layout
     for dma_gather via a DRAM round-trip; gather 2x2 patches per pixel with
     nc.gpsimd.dma_gather (768 B per index); weighted-sum the 4 corners
     (weights enter as step-0 broadcast APs), channel-dot, mask, transpose,
     store.
"""

import numpy as np

import concourse.bass as bass
import concourse.bacc as bacc
import concourse.mybir as mybir
import concourse.tile as tile
from concourse import bass_utils
from concourse.masks import make_identity

MD = 4
R = (2 * MD + 1) ** 2  # 81
B, C, H, W = 2, 48, 80, 128
HWPIX = H * W
SW = float(W) / float(W - 1)
SH = float(H) / float(H - 1)
NCORES = 8
NSB = 11          # slots per batch half (8*11 = 88 >= 81)
NSLOT = 2 * NSB   # 22
CH = 40           # h-chunk size (2 chunks per plane)
QW = 4 * C        # quad patch payload (192 elements)
TQW = QW          # table row width (f32, 768 B rows)

F32 = mybir.dt.float32
I32 = mybir.dt.int32
I16 = mybir.dt.int16
BF16 = mybir.dt.bfloat16
AF = mybir.ActivationFunctionType
OP = mybir.AluOpType

NSC = 4  # per-slot scalar columns: cfx, cbx, cfy, cby


def _plan():
    """Slots 0..NSB-1 are b=0 planes, NSB..2NSB-1 are b=1 planes."""
    counts = {0: [11, 10, 10, 10, 10, 10, 10, 10],
              1: [11, 10, 10, 10, 10, 10, 10, 10]}
    slots_per_core = []   # list of NSLOT (b, r)
    valid_per_core = []   # list of NSLOT bool
    for k in range(NCORES):
        slots, valid = [], []
        for b in (0, 1):
            start = sum(counts[b][:k])
            rs = list(range(start, start + counts[b][k]))
            v = [True] * len(rs)
            while len(rs) < NSB:
                rs.append(rs[-1])
                v.append(False)
            slots += [(b, r) for r in rs]
            valid += v
        slots_per_core.append(slots)
        valid_per_core.append(valid)
    return slots_per_core, valid_per_core


def _lin():
    return np.linspace(-MD, MD, 2 * MD + 1).astype(np.float64)


def build_program(dbg=False):
    nc = bacc.Bacc(
        "TRN2",
        target_bir_lowering=False,
        debug=False,
        enable_asserts=False,
        num_devices=NCORES,
        num_swdge_queues=2,
    )

    f1_d = nc.dram_tensor("f1", [B, C, H, W], F32, kind="ExternalInput")
    f2_d = nc.dram_tensor("f2", [B, C, H, W], F32, kind="ExternalInput")
    bmp_d = nc.dram_tensor("bmp", [NSLOT, 128, 2 * H], F32,
                           kind="ExternalInput")
    sc_d = nc.dram_tensor("sc", [128, NSLOT * NSC], F32, kind="ExternalInput")
    wio_d = nc.dram_tensor("wio", [128, 1], F32, kind="ExternalInput")
    hf_d = nc.dram_tensor("hf", [128, H], F32, kind="ExternalInput")
    out_d = nc.dram_tensor("out", [NSLOT, H, W], F32, kind="ExternalOutput")

    with tile.TileContext(nc) as tc:
        with (
            tc.tile_pool(name="const", bufs=1) as constp,
            tc.tile_pool(name="dram", bufs=1, space="DRAM") as dramp,
        ):
            ident = constp.tile([128, 128], F32)
            make_identity(nc, ident[:])
            eps = constp.tile([128, 1], F32)
            nc.gpsimd.memset(eps[:], 1e-6)
            wio = constp.tile([128, 1], F32)
            nc.sync.dma_start(out=wio[:], in_=wio_d.ap())
            hf = constp.tile([128, H], F32)
            nc.sync.dma_start(out=hf[:], in_=hf_d.ap())
            sc = constp.tile([128, NSLOT * NSC], F32)
            nc.sync.dma_start(out=sc[:], in_=sc_d.ap())

            # tabs[f][b]; f=0 -> feature1 (bw warp), f=1 -> feature2 (fw)
            t10 = dramp.tile([HWPIX, TQW], F32)
            t11 = dramp.tile([HWPIX, TQW], F32)
            t20 = dramp.tile([HWPIX, TQW], F32)
            t21 = dramp.tile([HWPIX, TQW], F32)
            tabs = [[t10, t11], [t20, t21]]

            # ---------------- Phase 1: normalize + quad tables -------------
            with (
                tc.tile_pool(name="fc", bufs=1) as fcp,
                tc.tile_pool(name="qt", bufs=1) as qtp,
                tc.tile_pool(name="ps", bufs=2, space="PSUM") as psp,
            ):
                WH = W // 2
                for tabi in range(4):
                    f = tabi // 2
                    b = tabi % 2
                    qt = qtp.tile([H, W, 4, C], F32, tag="qt")
                    for wh in range(2):  # w halves to bound SBUF
                        src = (f1_d if f == 0 else f2_d).ap()[b]
                        src = src[:, :, wh * WH:(wh + 1) * WH]  # [C, H, WH]
                        fc = fcp.tile([C, H, WH], F32, tag="fc")
                        nc.sync.dma_start(out=fc[:], in_=src)
                        for j in range(WH // 8):
                            pt = psp.tile([H, 8 * C], F32, tag="pt")
                            for jj in range(8):
                                w = 8 * j + jj
                                nc.tensor.transpose(
                                    out=pt[:, jj * C:(jj + 1) * C],
                                    in_=fc[:, :, w],
                                    identity=ident[:C, :C],
                                )
                            wg = wh * WH + 8 * j
                            if j % 2 == 0:
                                nc.vector.tensor_copy(
                                    qt[:, wg:wg + 8, 0, :], pt[:])
                            else:
                                nc.scalar.copy(
                                    qt[:, wg:wg + 8, 0, :], pt[:])

                    # normalize over c: squares into qt slot 1 (scratch)
                    nc.scalar.activation(
                        qt[:, :, 1, :], qt[:, :, 0, :], AF.Square)
                    ssq = fcp.tile([H, W], F32, tag="ssq")
                    nc.vector.tensor_reduce(
                        ssq[:], qt[:, :, 1, :], axis=mybir.AxisListType.X,
                        op=OP.add)
                    rn = fcp.tile([H, W], F32, tag="rn")
                    nc.scalar.activation(
                        rn[:], ssq[:], AF.Sqrt, bias=eps[:H, :])
                    nc.vector.reciprocal(rn[:], rn[:])
                    nc.scalar.copy(
                        qt[:, :, 1, :],
                        rn[:].unsqueeze(-1).broadcast_to([H, W, C]))
                    nc.vector.tensor_mul(
                        qt[:, :, 0, :], qt[:, :, 0, :], qt[:, :, 1, :])

                    # x-shift into slot 1 (from normalized slot 0)
                    nc.scalar.copy(qt[:, 0:W - 1, 1, :], qt[:, 1:W, 0, :])
                    nc.scalar.copy(qt[:, W - 1, 1, :], qt[:, W - 1, 0, :])
                    # y-shift via SBUF->SBUF DMA (partition shift)
                    nc.sync.dma_start(
                        out=qt[0:H - 1, :, 2:4, :], in_=qt[1:H, :, 0:2, :])
                    nc.sync.dma_start(
                        out=qt[H - 1:H, :, 2:4, :], in_=qt[H - 1:H, :, 0:2, :])

                    # write table rows [HWPIX, QW]
                    dst = tabs[f][b][:]
                    dst = dst.rearrange("(h w) q -> h (w q)", h=H)
                    nc.sync.dma_start(
                        out=dst, in_=qt[:].rearrange("h w a c -> h (w a c)"))

            # ---------------- Phase 2: per-plane slots ----------------------
            with (
                tc.tile_pool(name="fld", bufs=2) as fld,
                tc.tile_pool(name="pre", bufs=3) as prep,
                tc.tile_pool(name="iscr", bufs=2, space="DRAM") as iscrp,
                tc.tile_pool(name="ops", bufs=2, space="PSUM") as psp2,
            ):
                for s in range(NSLOT):
                    sb = 0 if s < NSB else 1
                    tabF = tabs[1][sb][:]   # f2 quad table
                    tabB = tabs[0][sb][:]   # f1 quad table
                    col = lambda j: sc[:, s * NSC + j:s * NSC + j + 1]
                    cfx, cbx, cfy, cby = (col(j) for j in range(NSC))

                    bmp = fld.tile([128, 2 * H], F32, tag="bmp")
                    nc.sync.dma_start(out=bmp[:], in_=bmp_d.ap()[s])
                    bmx = bmp[:, 0:H]
                    bmy = bmp[:, H:2 * H]

                    # sample coordinates, both warps batched in one
                    # [128, 2H] field: cols 0:H warp F, H:2H warp B
                    H2 = 2 * H
                    t2 = lambda tg: fld.tile([128, H2], F32, tag=tg, name=tg)
                    ix2 = t2("ix2")
                    nc.vector.tensor_scalar(
                        out=ix2[:, 0:H], in0=bmx, scalar1=wio[:], scalar2=SW,
                        op0=OP.add, op1=OP.mult)
                    nc.vector.tensor_scalar(
                        out=ix2[:, 0:H], in0=ix2[:, 0:H], scalar1=cfx,
                        scalar2=None, op0=OP.add)
                    nc.vector.tensor_scalar(
                        out=ix2[:, H:H2], in0=bmx, scalar1=wio[:],
                        scalar2=-SW, op0=OP.subtract, op1=OP.mult)
                    nc.vector.tensor_scalar(
                        out=ix2[:, H:H2], in0=ix2[:, H:H2], scalar1=cbx,
                        scalar2=None, op0=OP.add)
                    iy2 = t2("iy2")
                    tmy = fld.tile([128, H], F32, tag="tmy")
                    nc.vector.tensor_add(tmy[:], bmy, hf[:])
                    nc.vector.tensor_scalar(
                        out=iy2[:, 0:H], in0=tmy[:], scalar1=SH, scalar2=cfy,
                        op0=OP.mult, op1=OP.add)
                    nc.vector.tensor_sub(tmy[:], hf[:], bmy)
                    nc.vector.tensor_scalar(
                        out=iy2[:, H:H2], in0=tmy[:], scalar1=SH, scalar2=cby,
                        op0=OP.mult, op1=OP.add)

                    ixc = t2("ixc")
                    nc.vector.tensor_scalar(
                        out=ixc[:], in0=ix2[:], scalar1=0.0,
                        scalar2=float(W - 1), op0=OP.max, op1=OP.min)
                    iyc = t2("iyc")
                    nc.vector.tensor_scalar(
                        out=iyc[:], in0=iy2[:], scalar1=0.0,
                        scalar2=float(H - 1), op0=OP.max, op1=OP.min)

                    def floorfrac(srcf, f0tag, frtag):
                        # floor for srcf >= 0, robust to convert rounding
                        xi = fld.tile([128, H2], I32, tag=f0tag + "i",
                                      name=f0tag + "i")
                        nc.vector.tensor_copy(xi[:], srcf[:])
                        xf = t2(f0tag + "f")
                        nc.vector.tensor_copy(xf[:], xi[:])
                        er = t2(f0tag + "e")
                        nc.vector.tensor_tensor(
                            out=er[:], in0=xf[:], in1=srcf[:], op=OP.is_gt)
                        f0 = t2(f0tag)
                        nc.vector.tensor_sub(f0[:], xf[:], er[:])
                        fr = t2(frtag)
                        nc.vector.tensor_sub(fr[:], srcf[:], f0[:])
                        return f0, fr

                    x0, wx = floorfrac(ixc, "x0", "wx")
                    y0, wy = floorfrac(iyc, "y0", "wy")
                    xcf = t2("xcf")
                    nc.vector.tensor_scalar(
                        out=xcf[:], in0=x0[:], scalar1=float(W - 2),
                        scalar2=None, op0=OP.min)
                    bx = t2("bx")
                    nc.vector.tensor_sub(bx[:], x0[:], xcf[:])
                    nc.vector.tensor_add(bx[:], bx[:], wx[:])
                    # gather row index = y0*W + xc  (fits int16)
                    idxf = t2("idxf")
                    nc.vector.scalar_tensor_tensor(
                        out=idxf[:], in0=y0[:], scalar=float(W), in1=xcf[:],
                        op0=OP.mult, op1=OP.add)
                    # mask (trapezoid per axis)
                    ma = t2("ma")
                    nc.vector.tensor_scalar(
                        out=ma[:], in0=ix2[:], scalar1=-1.0,
                        scalar2=float(W), op0=OP.mult, op1=OP.add)
                    mb = t2("mb")
                    nc.vector.tensor_scalar(
                        out=mb[:], in0=ix2[:], scalar1=1.0, scalar2=None,
                        op0=OP.add)
                    nc.vector.tensor_tensor(
                        out=ma[:], in0=ma[:], in1=mb[:], op=OP.min)
                    nc.vector.tensor_scalar(
                        out=ma[:], in0=ma[:], scalar1=0.0, scalar2=1.0,
                        op0=OP.max, op1=OP.min)
                    mc = t2("mc")
                    nc.vector.tensor_scalar(
                        out=mc[:], in0=iy2[:], scalar1=-1.0,
                        scalar2=float(H), op0=OP.mult, op1=OP.add)
                    md = t2("md")
                    nc.vector.tensor_scalar(
                        out=md[:], in0=iy2[:], scalar1=1.0, scalar2=None,
                        op0=OP.add)
                    nc.vector.tensor_tensor(
                        out=mc[:], in0=mc[:], in1=md[:], op=OP.min)
                    nc.vector.tensor_scalar(
                        out=mc[:], in0=mc[:], scalar1=0.0, scalar2=1.0,
                        op0=OP.max, op1=OP.min)
                    msk2 = t2("msk2")
                    nc.vector.tensor_mul(msk2[:], ma[:], mc[:])
                    # bilinear corner weights
                    uy = t2("uy")
                    nc.vector.tensor_scalar(
                        out=uy[:], in0=wy[:], scalar1=-1.0, scalar2=1.0,
                        op0=OP.mult, op1=OP.add)
                    vx = t2("vx")
                    nc.vector.tensor_scalar(
                        out=vx[:], in0=bx[:], scalar1=-1.0, scalar2=1.0,
                        op0=OP.mult, op1=OP.add)
                    wa = t2("wa")
                    nc.vector.tensor_mul(wa[:], uy[:], vx[:])
                    wb = t2("wb")
                    nc.vector.tensor_mul(wb[:], uy[:], bx[:])
                    wc = t2("wc")
                    nc.vector.tensor_mul(wc[:], wy[:], vx[:])
                    wd = t2("wd")
                    nc.vector.tensor_mul(wd[:], wy[:], bx[:])

                    # per-warp: int16 idx + wrapped layout via DRAM trip
                    wrs = []
                    for u, wtag in enumerate(("F", "Bw")):
                        idx16 = fld.tile([128, H], I16, tag=wtag + "idx16",
                                         name=wtag + "idx16")
                        nc.vector.tensor_copy(
                            idx16[:], idxf[:, u * H:(u + 1) * H])
                        iscr = iscrp.tile([16, 8 * H], I16,
                                          tag=wtag + "iscr",
                                          name=wtag + "iscr")
                        nc.sync.dma_start(
                            out=iscr[:].rearrange("q (h m) -> m q h", m=8),
                            in_=idx16[:])
                        wr = fld.tile([128, 8 * H], I16, tag=wtag + "wr",
                                      name=wtag + "wr")
                        nc.sync.dma_start(
                            out=wr[:],
                            in_=iscr[:].unsqueeze(0).broadcast_to(
                                [8, 16, 8 * H]))
                        wrs.append(wr)
                    wrF, wrB = wrs
                    wgt2 = (wa, wb, wc, wd)
                    wF = tuple(w[:, 0:H] for w in wgt2)
                    wB = tuple(w[:, H:H2] for w in wgt2)

                    mall = fld.tile([128, H], F32, tag="mall")
                    nc.vector.tensor_mul(
                        mall[:], msk2[:, 0:H], msk2[:, H:H2])
                    nc.vector.tensor_scalar(
                        out=mall[:], in0=mall[:], scalar1=0.999, scalar2=None,
                        op0=OP.is_ge)

                    acc = fld.tile([128, H], F32, tag="acc")

                    for c0 in range(0, H, CH):
                        pres = []
                        for wi, (wrt, tabt, wgt) in enumerate(
                                ((wrF, tabF, wF), (wrB, tabB, wB))):
                            eng = nc.vector if wi == 0 else nc.gpsimd
                            pre = prep.tile(
                                [128, CH, TQW], F32, tag="pre",
                                name=f"pre{wi}")
                            off = c0 * 8
                            nc.gpsimd.dma_gather(
                                out_ap=pre[:],
                                in_ap=tabt,
                                idxs_ap=wrt[:, off:off + CH * 8],
                                num_idxs=CH * 128,
                                num_idxs_reg=CH * 128,
                                elem_size=TQW,
                                single_packet=False,
                                queue_num=wi,
                            )
                            # weighted sum of 4 corners (in place); weights
                            # enter as step-0 broadcast APs over channels.
                            for q in range(4):
                                wq = wgt[q][:, c0:c0 + CH]
                                wq = wq.unsqueeze(-1).broadcast_to(
                                    [128, CH, C])
                                eng.tensor_mul(
                                    pre[:, :, q * C:(q + 1) * C],
                                    pre[:, :, q * C:(q + 1) * C], wq)
                            eng.tensor_add(
                                pre[:, :, 0:2 * C], pre[:, :, 0:2 * C],
                                pre[:, :, 2 * C:4 * C])
                            eng.tensor_add(
                                pre[:, :, 0:C], pre[:, :, 0:C],
                                pre[:, :, C:2 * C])
                            pres.append(pre)
                        # channel dot
                        nc.vector.tensor_mul(
                            pres[0][:, :, 0:C], pres[0][:, :, 0:C],
                            pres[1][:, :, 0:C])
                        nc.vector.tensor_reduce(
                            acc[:, c0:c0 + CH], pres[0][:, :, 0:C],
                            axis=mybir.AxisListType.X, op=OP.add)

                    nc.vector.tensor_mul(acc[:], acc[:], mall[:])

                    # transpose [128, 80] -> [80, 128] and store
                    pt2 = psp2.tile([H, 128], F32, tag="pt2")
                    nc.tensor.transpose(
                        out=pt2[:], in_=acc[:], identity=ident[:])
                    ot = fld.tile([H, W], F32, tag="ot")
                    nc.scalar.copy(ot[:], pt2[:])
                    nc.sync.dma_start(out=out_d.ap()[s], in_=ot[:])

    nc.compile()
    return nc


def make_in_maps(feature1, feature2, BM):
    """Build the 8 per-core input maps from full inputs."""
    slots_per_core, valid_per_core = _plan()
    lin = _lin()
    f1 = np.ascontiguousarray(np.asarray(feature1, dtype=np.float32))
    f2 = np.ascontiguousarray(np.asarray(feature2, dtype=np.float32))
    bm = np.asarray(BM, dtype=np.float32)

    wio = np.arange(W, dtype=np.float32).reshape(128, 1)
    hfv = np.broadcast_to(
        np.arange(H, dtype=np.float32)[None, :], (128, H)).copy()

    in_maps = []
    for k in range(NCORES):
        slots = slots_per_core[k]
        bmp = np.zeros((NSLOT, 128, 2 * H), np.float32)
        sc = np.zeros((128, NSLOT * NSC), np.float32)
        for s, (b, r) in enumerate(slots):
            du = lin[r % (2 * MD + 1)]
            dv = lin[r // (2 * MD + 1)]
            bmp[s, :, 0:H] = bm[b, 0].T  # [w, h]
            bmp[s, :, H:2 * H] = bm[b, 1].T
            sc[:, s * NSC + 0] = np.float32(du * SW - 0.5)   # cfx
            sc[:, s * NSC + 1] = np.float32(-du * SW - 0.5)  # cbx
            sc[:, s * NSC + 2] = np.float32(dv * SH - 0.5)   # cfy
            sc[:, s * NSC + 3] = np.float32(-dv * SH - 0.5)  # cby
        in_maps.append({
            "f1": f1, "f2": f2,
            "bmp": bmp, "sc": sc,
            "wio": wio, "hf": hfv,
        })
    return in_maps, slots_per_core, valid_per_core


_NC_CACHE = {}


def get_program():
    if "nc" not in _NC_CACHE:
        _NC_CACHE["nc"] = build_program()
    return _NC_CACHE["nc"]


def assemble_output(results, slots_per_core, valid_per_core):
    out = np.zeros((B, R, H, W), np.float32)
    for k in range(NCORES):
        core_out = results[k]["out"]  # [NSLOT, H, W]
        for s in range(NSLOT):
            if valid_per_core[k][s]:
                b, r = slots_per_core[k][s]
                out[b, r] = core_out[s]
    return out


def kernel(feature1, feature2, BM):
    nc = get_program()
    in_maps, slots_per_core, valid_per_core = make_in_maps(
        feature1, feature2, BM)
    res = bass_utils.run_bass_kernel_spmd(
        nc, in_maps, core_ids=list(range(NCORES)))
    return assemble_output(res.results, slots_per_core, valid_per_core)



# revision 3
# speedup vs baseline: 2.4486x; 2.4486x over previous
"""Bass/TRN2 kernel for nn_BilateralCostVolume — patch-gather scheme.

Sharding: core k handles batch b = k//4, output rows h in [20*(k%4), +20).
Per core, per pixel, per warp (F: +displacement on f2n; B: -displacement on
f1n) gather an 11x11 patch (11 descriptors of 11-col rows) from a DRAM table
whose rows are overlapping 11-col windows of the padded normalized feature.
All 81 displacements are then computed on-chip with static 3-tap separable
interpolation (carry folded into per-pixel weights), channel dot, mask.

out[b, r, h0+hh, w] = core_out[w, hh*81 + r].
"""

import numpy as np

import concourse.bass as bass
import concourse.bacc as bacc
import concourse.mybir as mybir
import concourse.tile as tile
from concourse import bass_utils

B_, C, H, W = 2, 48, 80, 128
R = 81
ND = 9
MD = 4
SW = W / (W - 1.0)
SH = H / (H - 1.0)
TH_X = 4.0 * (SW - 1.0)
TH_Y = 4.0 * (SH - 1.0)
NCORES = 8
NRB = 20            # output rows per core
PADL = 10
NCOLS = 11          # cols per table row
N_XS = 138          # x starts
Wp = 148            # padded width
NY = 40             # table y rows
NK = 11             # patch rows gathered per pixel
ELEM = 640          # elems per table row (bf16): 528 used + pad (1280 B)
NPX = 6016          # padded pixel rows in fp dram (40*148=5920 -> 47*128)
NTROW = NY * N_XS   # 5520 table rows
NIDX = NK * 128     # 1408 idxs per gather
TSLOT = NIDX // 16  # 88

F32 = mybir.dt.float32
I32 = mybir.dt.int32
I16 = mybir.dt.int16
BF16 = mybir.dt.bfloat16
AF = mybir.ActivationFunctionType
OP = mybir.AluOpType
LIN = np.linspace(-MD, MD, ND)

# engine assignment per warp: units running muls on ACT (adds on DVE), and
# units running fully on Pool (ts + 2x scalar_tensor_tensor)
ACT_FIRST = (1, 7)
POOL_FIRST = (4,)
ACT_SECOND = (1, 7)
POOL_SECOND = (4,)


def mkap(t, dims, offset_elems=0):
    """Overlapping/custom AP on a dram tensor: dims = [[stride, count], ...]."""
    import bass_rust
    a = t.ap().copy() if hasattr(t, "ap") else t.copy()
    a.ap = bass_rust.VecI64Pair([list(d) for d in dims])
    if offset_elems:
        a.offset = a.offset + offset_elems
    return a


def emit_unit(nc, tp, n, eng, ins, ws, outap):
    """outap = ws[0]*ins[0] + ws[1]*ins[1] + ws[2]*ins[2] on the given
    engine ('dve' | 'act' = ACT muls + DVE adds | 'pool' = ts + 2 stt)."""
    t0 = tp.tile([128, n], BF16, tag=f"t0_{n}", name=f"t0_{n}")
    if eng == "pool":
        nc.gpsimd.tensor_scalar(
            out=t0[:], in0=ins[0], scalar1=ws[0], scalar2=None, op0=OP.mult)
        nc.gpsimd.scalar_tensor_tensor(
            out=t0[:], in0=ins[1], scalar=ws[1], in1=t0[:],
            op0=OP.mult, op1=OP.add)
        nc.gpsimd.scalar_tensor_tensor(
            out=outap, in0=ins[2], scalar=ws[2], in1=t0[:],
            op0=OP.mult, op1=OP.add)
        return
    t1 = tp.tile([128, n], BF16, tag=f"t1_{n}", name=f"t1_{n}")
    if eng == "act":
        nc.scalar.activation(t0[:], ins[0], AF.Copy, scale=ws[0])
        nc.scalar.activation(t1[:], ins[1], AF.Copy, scale=ws[1])
        nc.vector.tensor_add(t0[:], t0[:], t1[:])
        nc.scalar.activation(t1[:], ins[2], AF.Copy, scale=ws[2])
        nc.vector.tensor_add(outap, t0[:], t1[:])
    else:
        nc.vector.tensor_scalar(
            out=t0[:], in0=ins[0], scalar1=ws[0], scalar2=None, op0=OP.mult)
        nc.vector.tensor_scalar(
            out=t1[:], in0=ins[1], scalar1=ws[1], scalar2=None, op0=OP.mult)
        nc.vector.tensor_add(t0[:], t0[:], t1[:])
        nc.vector.tensor_scalar(
            out=t1[:], in0=ins[2], scalar1=ws[2], scalar2=None, op0=OP.mult)
        nc.vector.tensor_add(outap, t0[:], t1[:])


def build_program():
    nc = bacc.Bacc(
        "TRN2",
        target_bir_lowering=False,
        debug=False,
        enable_asserts=False,
        num_devices=NCORES,
        num_swdge_queues=2,
    )

    f1s_d = nc.dram_tensor("f1s", [NPX, C], F32, kind="ExternalInput")
    f2s_d = nc.dram_tensor("f2s", [NPX, C], F32, kind="ExternalInput")
    bmx_d = nc.dram_tensor("bmx", [128, NRB], F32, kind="ExternalInput")
    bmy_d = nc.dram_tensor("bmy", [128, NRB], F32, kind="ExternalInput")
    wio_d = nc.dram_tensor("wio", [128, 1], F32, kind="ExternalInput")
    hcon_d = nc.dram_tensor("hcon", [128, NRB], F32, kind="ExternalInput")
    y0con_d = nc.dram_tensor("y0con", [128, 1], F32, kind="ExternalInput")
    # per-warp du/dv constant rows: g (frac inc), mg (mask pos inc), krow
    gx_d = nc.dram_tensor("gx", [128, 2 * ND], F32, kind="ExternalInput")
    gy_d = nc.dram_tensor("gy", [128, 2 * ND], F32, kind="ExternalInput")
    mgx_d = nc.dram_tensor("mgx", [128, 2 * ND], F32, kind="ExternalInput")
    mgy_d = nc.dram_tensor("mgy", [128, 2 * ND], F32, kind="ExternalInput")
    krow_d = nc.dram_tensor("krow", [128, NK], F32, kind="ExternalInput")

    fp1_d = nc.dram_tensor("fp1", [NPX, C], BF16, kind="Internal")
    fp2_d = nc.dram_tensor("fp2", [NPX, C], BF16, kind="Internal")
    tab1_d = nc.dram_tensor("tab1", [NTROW, ELEM], BF16, kind="Internal")
    tab2_d = nc.dram_tensor("tab2", [NTROW, ELEM], BF16, kind="Internal")
    iscr_d = nc.dram_tensor("iscr", [2, 16, NRB * TSLOT], I16, kind="Internal")
    out_d = nc.dram_tensor("out", [128, NRB * R], F32, kind="ExternalOutput")

    with tile.TileContext(nc) as tc:
        with tc.tile_pool(name="const", bufs=1) as constp:
            eps = constp.tile([128, 1], F32)
            nc.gpsimd.memset(eps[:], 1e-6)
            wio = constp.tile([128, 1], F32)
            nc.sync.dma_start(out=wio[:], in_=wio_d.ap())
            hcon = constp.tile([128, NRB], F32)
            nc.sync.dma_start(out=hcon[:], in_=hcon_d.ap())
            y0con = constp.tile([128, 1], F32)
            nc.sync.dma_start(out=y0con[:], in_=y0con_d.ap())
            gx = constp.tile([128, 2 * ND], F32)
            nc.sync.dma_start(out=gx[:], in_=gx_d.ap())
            gy = constp.tile([128, 2 * ND], F32)
            nc.sync.dma_start(out=gy[:], in_=gy_d.ap())
            mgx = constp.tile([128, 2 * ND], F32)
            nc.sync.dma_start(out=mgx[:], in_=mgx_d.ap())
            mgy = constp.tile([128, 2 * ND], F32)
            nc.sync.dma_start(out=mgy[:], in_=mgy_d.ap())
            krow = constp.tile([128, NK], F32)
            nc.sync.dma_start(out=krow[:], in_=krow_d.ap())
            bmx = constp.tile([128, NRB], F32)
            nc.sync.dma_start(out=bmx[:], in_=bmx_d.ap())
            bmy = constp.tile([128, NRB], F32)
            nc.sync.dma_start(out=bmy[:], in_=bmy_d.ap())

            # ------------ Phase A: normalize features -> fp dram (bf16) -----
            with tc.tile_pool(name="norm", bufs=1) as normp:
                for fsrc, fdst in ((f1s_d, fp1_d), (f2s_d, fp2_d)):
                    ld = normp.tile([128, 47, C], F32, tag="ld")
                    src = mkap(fsrc, [[C, 128], [128 * C, 47], [1, C]])
                    nc.sync.dma_start(out=ld[:], in_=src)
                    sq = normp.tile([128, 47, C], F32, tag="sq")
                    nc.vector.tensor_mul(sq[:], ld[:], ld[:])
                    ssq = normp.tile([128, 47], F32, tag="ssq")
                    nc.vector.tensor_reduce(
                        ssq[:], sq[:], axis=mybir.AxisListType.X, op=OP.add)
                    nc.scalar.activation(ssq[:], ssq[:], AF.Sqrt, bias=eps[:])
                    nc.vector.reciprocal(ssq[:], ssq[:])
                    nf = normp.tile([128, 47, C], BF16, tag="nf")
                    nc.vector.tensor_mul(
                        nf[:], ld[:],
                        ssq[:].unsqueeze(-1).broadcast_to([128, 47, C]))
                    dst = mkap(fdst, [[C, 128], [128 * C, 47], [1, C]])
                    nc.sync.dma_start(out=dst, in_=nf[:])

            # ------------ Phase A2: table build (overlap-window DMAs) -------
            for fp, tab in ((fp2_d, tab1_d), (fp1_d, tab2_d)):
                # tab1 <- f2 (forward warp), tab2 <- f1 (backward)
                for xo, cnt in ((0, 128), (128, N_XS - 128)):
                    src = mkap(fp, [[C, cnt], [Wp * C, NY], [1, NCOLS * C]],
                               offset_elems=xo * C)
                    dst = mkap(tab, [[ELEM, cnt], [N_XS * ELEM, NY],
                                     [1, NCOLS * C]],
                               offset_elems=xo * ELEM)
                    nc.sync.dma_start(out=dst, in_=src)

            # ------------ Phase B: fields ----------------------------------
            # per warp (F=0 sgn +1, B=1 sgn -1): weights, masks, indices
            wA = []   # wA[warp][axis][tap] -> [128, NRB, ND] f32
            fldcm = tc.tile_pool(name="fld", bufs=1)
            fldp = fldcm.__enter__()
            maskC = fldp.tile([128, NRB, R], F32)

            idx16 = []
            scrcm = tc.tile_pool(name="scr", bufs=1)
            scrp = scrcm.__enter__()
            for wi, sgn in ((0, 1.0), (1, -1.0)):
                vx = scrp.tile([128, NRB], F32, tag=f"vx{wi}", name=f"vx{wi}")
                nc.vector.tensor_scalar(
                    out=vx[:], in0=bmx[:], scalar1=sgn, scalar2=wio[:],
                    op0=OP.mult, op1=OP.add)
                nc.vector.tensor_scalar(
                    out=vx[:], in0=vx[:], scalar1=float(SW), scalar2=-0.5,
                    op0=OP.mult, op1=OP.add)
                vy = scrp.tile([128, NRB], F32, tag=f"vy{wi}", name=f"vy{wi}")
                nc.vector.tensor_scalar(
                    out=vy[:], in0=bmy[:], scalar1=sgn, scalar2=None,
                    op0=OP.mult)
                nc.vector.tensor_add(vy[:], vy[:], hcon[:])
                nc.vector.tensor_scalar(
                    out=vy[:], in0=vy[:], scalar1=float(SH), scalar2=-0.5,
                    op0=OP.mult, op1=OP.add)

                axes = []
                bases = []
                for ax, (v, th, gt) in enumerate(
                        ((vx, TH_X, gx), (vy, TH_Y, gy))):
                    pfx = f"w{wi}a{ax}"
                    t2 = lambda tg: scrp.tile([128, NRB], F32,
                                              tag=pfx + tg, name=pfx + tg)
                    xi = scrp.tile([128, NRB], I32, tag=pfx + "i",
                                   name=pfx + "i")
                    nc.vector.tensor_copy(xi[:], v[:])
                    xf = t2("xf")
                    nc.vector.tensor_copy(xf[:], xi[:])
                    er = t2("er")
                    nc.vector.tensor_tensor(
                        out=er[:], in0=xf[:], in1=v[:], op=OP.is_gt)
                    base = t2("b")
                    nc.vector.tensor_sub(base[:], xf[:], er[:])
                    fx = t2("fx")
                    nc.vector.tensor_sub(fx[:], v[:], base[:])
                    sig = t2("sg")
                    nc.vector.tensor_scalar(
                        out=sig[:], in0=fx[:], scalar1=float(th),
                        scalar2=None, op0=OP.is_lt)
                    # phi/ep/om over [128, NRB, ND]
                    t3 = lambda tg: scrp.tile([128, NRB, ND], F32,
                                              tag=pfx + tg, name=pfx + tg)
                    gb = gt[:, wi * ND:(wi + 1) * ND]
                    gbb = gb.unsqueeze(1).broadcast_to([128, NRB, ND])
                    fxb = fx[:].unsqueeze(-1).broadcast_to([128, NRB, ND])
                    sgb = sig[:].unsqueeze(-1).broadcast_to([128, NRB, ND])
                    phi = t3("ph")
                    nc.vector.tensor_tensor(
                        out=phi[:], in0=fxb, in1=gbb, op=OP.add)
                    thr = t2("th")
                    nc.vector.tensor_scalar(
                        out=thr[:], in0=sig[:], scalar1=-1.0, scalar2=1.0,
                        op0=OP.mult, op1=OP.add)
                    ep = t3("ep")
                    nc.vector.tensor_tensor(
                        out=ep[:], in0=phi[:],
                        in1=thr[:].unsqueeze(-1).broadcast_to([128, NRB, ND]),
                        op=OP.is_ge)
                    om = t3("om")
                    nc.vector.tensor_sub(om[:], phi[:], ep[:])
                    nc.vector.tensor_tensor(
                        out=om[:], in0=om[:], in1=sgb, op=OP.add)
                    # A0 = (1-ep)(1-om), A1 = ep+om-2ep*om, A2 = ep*om
                    A2 = fldp.tile([128, NRB, ND], F32, tag=pfx + "A2",
                                   name=pfx + "A2")
                    nc.vector.tensor_mul(A2[:], ep[:], om[:])
                    s = t3("s")
                    nc.vector.tensor_add(s[:], ep[:], om[:])
                    A1 = fldp.tile([128, NRB, ND], F32, tag=pfx + "A1",
                                   name=pfx + "A1")
                    nc.vector.scalar_tensor_tensor(
                        out=A1[:], in0=A2[:], scalar=-2.0, in1=s[:],
                        op0=OP.mult, op1=OP.add)
                    A0 = fldp.tile([128, NRB, ND], F32, tag=pfx + "A0",
                                   name=pfx + "A0")
                    nc.vector.scalar_tensor_tensor(
                        out=A0[:], in0=s[:], scalar=-1.0, in1=A2[:],
                        op0=OP.mult, op1=OP.add)
                    nc.vector.tensor_scalar(
                        out=A0[:], in0=A0[:], scalar1=1.0, scalar2=None,
                        op0=OP.add)
                    axes.append((A0, A1, A2))
                    bases.append((base, sig))
                wA.append(axes)

                # gather row indices
                (bx, sx), (by, sy) = bases[0], bases[1]
                sxf = scrp.tile([128, NRB], F32, tag=f"sx{wi}",
                                name=f"sx{wi}")
                nc.vector.tensor_sub(sxf[:], bx[:], sx[:])
                syf = scrp.tile([128, NRB], F32, tag=f"sy{wi}",
                                name=f"sy{wi}")
                nc.vector.tensor_sub(syf[:], by[:], sy[:])
                # row0 = (sy - 4)*138 - y0con*138 + sx - 4 + 10
                r0 = scrp.tile([128, NRB], F32, tag=f"r0{wi}", name=f"r0{wi}")
                nc.vector.tensor_scalar(
                    out=r0[:], in0=syf[:], scalar1=float(N_XS),
                    scalar2=y0con[:], op0=OP.mult, op1=OP.subtract)
                nc.vector.tensor_tensor(
                    out=r0[:], in0=r0[:], in1=sxf[:], op=OP.add)
                nc.vector.tensor_scalar(
                    out=r0[:], in0=r0[:],
                    scalar1=float(PADL - 4 - 4 * N_XS), scalar2=None,
                    op0=OP.add)
                idxf = scrp.tile([128, NRB, NK], F32, tag=f"ixf{wi}",
                                 name=f"ixf{wi}")
                nc.vector.tensor_tensor(
                    out=idxf[:],
                    in0=r0[:].unsqueeze(-1).broadcast_to([128, NRB, NK]),
                    in1=krow[:].unsqueeze(1).broadcast_to([128, NRB, NK]),
                    op=OP.add)
                ix16 = scrp.tile([128, NRB, NK], I16, tag=f"ix16{wi}",
                                 name=f"ix16{wi}")
                nc.vector.tensor_copy(ix16[:], idxf[:])
                idx16.append(ix16)

                # masks for this warp -> multiply into maskC
                mx = scrp.tile([128, NRB, ND], F32, tag=f"mx{wi}",
                               name=f"mx{wi}")
                my = scrp.tile([128, NRB, ND], F32, tag=f"my{wi}",
                               name=f"my{wi}")
                for (mt, v, mgt, lim) in ((mx, vx, mgx, float(W)),
                                          (my, vy, mgy, float(H))):
                    pos = scrp.tile([128, NRB, ND], F32, tag=f"pos{wi}",
                                    name=f"pos{wi}{lim}")
                    nc.vector.tensor_tensor(
                        out=pos[:],
                        in0=v[:].unsqueeze(-1).broadcast_to([128, NRB, ND]),
                        in1=mgt[:, wi * ND:(wi + 1) * ND].unsqueeze(1)
                            .broadcast_to([128, NRB, ND]),
                        op=OP.add)
                    t = scrp.tile([128, NRB, ND], F32, tag=f"mt{wi}",
                                  name=f"mt{wi}{lim}")
                    nc.vector.tensor_scalar(
                        out=t[:], in0=pos[:], scalar1=-1.0, scalar2=lim,
                        op0=OP.mult, op1=OP.add)
                    nc.vector.tensor_scalar(
                        out=pos[:], in0=pos[:], scalar1=1.0, scalar2=None,
                        op0=OP.add)
                    nc.vector.tensor_tensor(
                        out=t[:], in0=t[:], in1=pos[:], op=OP.min)
                    nc.vector.tensor_scalar(
                        out=mt[:], in0=t[:], scalar1=0.0, scalar2=1.0,
                        op0=OP.max, op1=OP.min)
                mw = scrp.tile([128, NRB, ND, ND], F32, tag=f"mw{wi}",
                               name=f"mw{wi}")
                nc.vector.tensor_tensor(
                    out=mw[:],
                    in0=my[:].unsqueeze(-1).broadcast_to([128, NRB, ND, ND]),
                    in1=mx[:].unsqueeze(2).broadcast_to([128, NRB, ND, ND]),
                    op=OP.mult)
                nc.vector.tensor_scalar(
                    out=mw[:], in0=mw[:], scalar1=0.999, scalar2=None,
                    op0=OP.is_ge)
                if wi == 0:
                    nc.vector.tensor_copy(
                        maskC[:], mw[:].rearrange("p n a b -> p n (a b)"))
                else:
                    nc.vector.tensor_mul(
                        maskC[:], maskC[:],
                        mw[:].rearrange("p n a b -> p n (a b)"))

            # idx wrap round trip
            wrs = []
            for wi in range(2):
                nc.sync.dma_start(
                    out=mkap(iscr_d,
                             [[1, 8], [NRB * TSLOT, 16], [TSLOT, NRB],
                              [8, NK]],
                             offset_elems=wi * 16 * NRB * TSLOT),
                    in_=idx16[wi][:])
                wr = fldp.tile([128, NRB * TSLOT], I16, tag=f"wr{wi}",
                               name=f"wr{wi}")
                src = iscr_d.ap()[wi]
                nc.sync.dma_start(
                    out=wr[:],
                    in_=src.unsqueeze(0).broadcast_to([8, 16, NRB * TSLOT]))
                wrs.append(wr)

            scrcm.__exit__(None, None, None)

            # ------------ Phase C: per-rb loop ------------------------------
            outacc = fldp.tile([128, NRB, R], F32)
            with (
                tc.tile_pool(name="pp", bufs=2) as pp,
                tc.tile_pool(name="pyp", bufs=2) as pyp,
                tc.tile_pool(name="fwp", bufs=2) as fwp,
                tc.tile_pool(name="tp", bufs=4) as tp,
                tc.tile_pool(name="dotp", bufs=1) as dotp,
            ):
                for rb in range(NRB):
                    fws = []
                    for wi, sgn in ((0, 1), (1, -1)):
                        tabt = (tab1_d if wi == 0 else tab2_d)
                        P = pp.tile([128, NK, ELEM], BF16, tag=f"P{wi}",
                                    name=f"P{wi}")
                        nc.gpsimd.dma_gather(
                            out_ap=P[:],
                            in_ap=tabt.ap(),
                            idxs_ap=wrs[wi][:, rb * TSLOT:(rb + 1) * TSLOT],
                            num_idxs=NIDX,
                            num_idxs_reg=NIDX,
                            elem_size=ELEM,
                            single_packet=False,
                            queue_num=wi,
                        )
                        (A0x, A1x, A2x), (A0y, A1y, A2y) = wA[wi]
                        # first pass: y taps -> Py[128, ND(dv), NCOLS, C]
                        Py = pyp.tile([128, ND, NCOLS, C], BF16,
                                      tag=f"Py{wi}", name=f"Py{wi}")
                        for idv in range(ND):
                            q = idv if wi == 0 else (ND - 1 - idv)
                            ws = (A0y[:, rb, idv:idv + 1],
                                  A1y[:, rb, idv:idv + 1],
                                  A2y[:, rb, idv:idv + 1])
                            ins = (P[:, q, 0:NCOLS * C],
                                   P[:, q + 1, 0:NCOLS * C],
                                   P[:, q + 2, 0:NCOLS * C])
                            outap = Py[:, idv, :, :].rearrange(
                                "p m c -> p (m c)")
                            emit_unit(nc, tp, NCOLS * C,
                                      "act" if idv in ACT_FIRST else
                                      "pool" if idv in POOL_FIRST else "dve",
                                      ins, ws, outap)
                        # second pass: x taps -> fw[128, ND(dv), ND(du), C]
                        fw = fwp.tile([128, ND, ND, C], BF16, tag=f"fw{wi}",
                                      name=f"fw{wi}")
                        for idu in range(ND):
                            q = idu if wi == 0 else (ND - 1 - idu)
                            ws = (A0x[:, rb, idu:idu + 1],
                                  A1x[:, rb, idu:idu + 1],
                                  A2x[:, rb, idu:idu + 1])
                            ins = (Py[:, :, q, :], Py[:, :, q + 1, :],
                                   Py[:, :, q + 2, :])
                            emit_unit(nc, tp, ND * C,
                                      "act" if idu in ACT_SECOND else
                                      "pool" if idu in POOL_SECOND else "dve",
                                      ins, ws, fw[:, :, idu, :])
                        fws.append(fw)
                    # dot + mask
                    fwF, fwB = fws
                    prod = fwF[:].rearrange("p a b c -> p (a b) c")
                    nc.vector.tensor_mul(
                        prod, prod, fwB[:].rearrange("p a b c -> p (a b) c"))
                    T1 = dotp.tile([128, R, C // 2], BF16, tag="T1",
                                   name="T1")
                    nc.vector.tensor_tensor(
                        out=T1[:], in0=prod[:, :, 0:24], in1=prod[:, :, 24:48],
                        op=OP.add)
                    T2 = dotp.tile([128, R, C // 4], BF16, tag="T2",
                                   name="T2")
                    nc.vector.tensor_tensor(
                        out=T2[:], in0=T1[:, :, 0:12], in1=T1[:, :, 12:24],
                        op=OP.add)
                    T3 = dotp.tile([128, R, C // 8], BF16, tag="T3",
                                   name="T3")
                    nc.vector.tensor_tensor(
                        out=T3[:], in0=T2[:, :, 0:6], in1=T2[:, :, 6:12],
                        op=OP.add)
                    nc.vector.tensor_reduce(
                        outacc[:, rb, :], T3[:], axis=mybir.AxisListType.X,
                        op=OP.add)
                    nc.vector.tensor_mul(
                        outacc[:, rb, :], outacc[:, rb, :], maskC[:, rb, :])

            nc.sync.dma_start(
                out=out_d.ap(),
                in_=outacc[:].rearrange("p n r -> p (n r)"))
            fldcm.__exit__(None, None, None)

    nc.compile()
    return nc


def make_in_maps(feature1, feature2, BM):
    f1 = np.asarray(feature1, dtype=np.float32)
    f2 = np.asarray(feature2, dtype=np.float32)
    bm = np.asarray(BM, dtype=np.float32)

    wio = np.arange(W, dtype=np.float32).reshape(128, 1)
    krow = np.broadcast_to(
        (np.arange(NK) * N_XS).astype(np.float32)[None, :], (128, NK)).copy()

    def padded_slice(f, b, h0):
        ys = np.clip(h0 - 10 + np.arange(NY), 0, H - 1)
        xs = np.clip(np.arange(Wp) - PADL, 0, W - 1)
        s = f[b][:, ys][:, :, xs]                 # [C, NY, Wp]
        s = np.ascontiguousarray(s.transpose(1, 2, 0)).reshape(NY * Wp, C)
        out = np.zeros((NPX, C), np.float32)
        out[:NY * Wp] = s
        return out

    mgx = np.zeros((128, 2 * ND), np.float32)
    mgy = np.zeros((128, 2 * ND), np.float32)
    gx = np.zeros((128, 2 * ND), np.float32)
    gy = np.zeros((128, 2 * ND), np.float32)
    d = LIN.astype(np.float64)
    for wi, sgn in ((0, 1.0), (1, -1.0)):
        gx[:, wi * ND:(wi + 1) * ND] = (sgn * d * (SW - 1.0)).astype(
            np.float32)[None, :]
        gy[:, wi * ND:(wi + 1) * ND] = (sgn * d * (SH - 1.0)).astype(
            np.float32)[None, :]
        mgx[:, wi * ND:(wi + 1) * ND] = (sgn * d * SW).astype(
            np.float32)[None, :]
        mgy[:, wi * ND:(wi + 1) * ND] = (sgn * d * SH).astype(
            np.float32)[None, :]

    in_maps = []
    for k in range(NCORES):
        b = k // 4
        h0 = 20 * (k % 4)
        hcon = np.broadcast_to(
            (h0 + np.arange(NRB)).astype(np.float32)[None, :],
            (128, NRB)).copy()
        y0con = np.full((128, 1), np.float32((h0 - 10) * N_XS), np.float32)
        in_maps.append({
            "f1s": padded_slice(f1, b, h0),
            "f2s": padded_slice(f2, b, h0),
            "bmx": np.ascontiguousarray(bm[b, 0, h0:h0 + NRB, :].T),
            "bmy": np.ascontiguousarray(bm[b, 1, h0:h0 + NRB, :].T),
            "wio": wio, "hcon": hcon, "y0con": y0con,
            "gx": gx, "gy": gy, "mgx": mgx, "mgy": mgy,
            "krow": krow,
        })
    return in_maps


_NC_CACHE = {}


def get_program():
    if "nc" not in _NC_CACHE:
        _NC_CACHE["nc"] = build_program()
    return _NC_CACHE["nc"]


def assemble_output(results):
    out = np.zeros((B_, R, H, W), np.float32)
    for k in range(NCORES):
        b = k // 4
        h0 = 20 * (k % 4)
        co = results[k]["out"].reshape(128, NRB, R)   # [w, hh, r]
        out[b, :, h0:h0 + NRB, :] = co.transpose(2, 1, 0)
    return out


def kernel(feature1, feature2, BM):
    nc = get_program()
    in_maps = make_in_maps(feature1, feature2, BM)
    res = bass_utils.run_bass_kernel_spmd(
        nc, in_maps, core_ids=list(range(NCORES)))
    return assemble_output(res.results)
